# revision 6
# baseline (speedup 1.0000x reference)
"""Trainium2 Bass kernel for chunked local self-attention (8-core SPMD).

Model (hardcoded from the problem spec):
  B=2, S=8192, HID=1024, NH=16, DH=64, CHUNK=64, N_BEFORE=1, N_AFTER=0,
  decoder-causal, softmax over a 128-wide rolled window per 64-chunk.

Sharding: sequence-parallel over 8 cores. Core i handles seq rows
[1024*i, 1024*(i+1)) of both batches, with a 128-row (2-chunk) front halo
(wrapped, matching jnp.roll semantics; the wrapped window is masked out
exactly as in the reference).

Per-core pipeline (per batch):
  1. DMA X slab [1152, 1024] fp32, PE-transpose to XT [hid, row] (f32r).
  2. QKV projections on PE in float32r (full speed at N>=256):
       QT[outd, row] (bf16), KT[outd, row]/8 (bf16), V[row, outd] (+ones col,
       bf16) via lhsT/rhs role swaps of XT.
  3. Attention per (512-row subpanel, head): band matmuls per 128-row V tile:
       PT_raw[kv, qi] = KT_tile^T-window x QT  (one MM per tile, N<=192)
       PT = exp(PT_raw) * mask   (ACT exp psum->bf16, DVE mask multiply)
       OT[65, 512] += [V|1]^T x PT   (PSUM accumulation across tiles; row 64
                                      accumulates the softmax denominators)
       O = transpose(OT) blocks, scale rows by 1/sums, DMA out.
"""

import sys

sys.path.insert(0, "/opt/trn_rl_repo")

import numpy as np
import ml_dtypes

B, S, HID = 2, 8192, 1024
NH, DH = 16, 64
CHUNK = 64
CORES = 8
SLICE = S // CORES          # 1024 q rows per core per batch
HALO = 128                  # 2-chunk front halo
SLAB = SLICE + HALO         # 1152
NRT = SLAB // 128           # 9 row tiles of V / X
NSP = SLICE // 512          # 2 attention subpanels per batch
KS = 384                    # KT projection free-dim span (>=256 for f32r)

_CACHE = {}


def _build():
    import concourse.bass as bass
    import concourse.tile as tile
    from concourse import mybir, bacc

    F32 = mybir.dt.float32
    F32R = mybir.dt.float32r
    BF16 = mybir.dt.bfloat16
    EXP = mybir.ActivationFunctionType.Exp

    nc = bacc.Bacc("TRN2", target_bir_lowering=False, debug=False,
                   num_devices=CORES)

    x = nc.dram_tensor("x", [B, SLAB, HID], F32, kind="ExternalInput")
    wq = nc.dram_tensor("wq", [HID, HID], F32R, kind="ExternalInput")
    wk = nc.dram_tensor("wk", [HID, HID], F32R, kind="ExternalInput")
    wv = nc.dram_tensor("wv", [HID, HID], F32R, kind="ExternalInput")
    mgen = nc.dram_tensor("mgen", [128, 320], BF16, kind="ExternalInput")
    mfirst = nc.dram_tensor("mfirst", [128, 128], BF16, kind="ExternalInput")
    ident = nc.dram_tensor("ident", [128, 128], F32, kind="ExternalInput")
    out = nc.dram_tensor("out", [B, SLICE, HID], F32, kind="ExternalOutput")

    # qi col spans (local to a 512-row subpanel) served by V-tile l = rt-4*sp.
    # Widened so the A-set (l=0,2,4) and B-set (l=1,3) each tile [0,512)
    # exactly: PSUM accumulation then always overwrites pending regions or
    # accumulates fully-written ones, and every byte of both accumulators is
    # written. The extra columns are zeroed by the mask.
    SPANS = [(0, 128), (0, 256), (128, 320), (256, 512), (320, 512)]
    # mask slice of mbig [128, 320] per l (see _masks)
    MSLICE = [(192, 320), (64, 320), (64, 256), (64, 320), (0, 192)]

    with tile.TileContext(nc) as tc:
        with (
            tc.tile_pool(name="big", bufs=1) as big,
            tc.tile_pool(name="xin", bufs=3) as xin_pool,
            tc.tile_pool(name="wqk", bufs=12) as wqk_pool,
            tc.tile_pool(name="wvp", bufs=10) as wv_pool,
            tc.tile_pool(name="pt", bufs=12) as pt_pool,
            tc.tile_pool(name="ots", bufs=3) as ots_pool,
            tc.tile_pool(name="osb", bufs=6) as osb_pool,
            tc.tile_pool(name="misc", bufs=1) as misc,
            tc.tile_pool(name="pss", bufs=6, space="PSUM") as ps_small,
            tc.tile_pool(name="psp", bufs=2, space="PSUM") as ps_proj,
        ):
            ident_sb = misc.tile([128, 128], F32, tag="ident")
            nc.sync.dma_start(out=ident_sb[:], in_=ident[:])
            mgen_sb = misc.tile([128, 320], BF16, tag="mgen")
            nc.sync.dma_start(out=mgen_sb[:], in_=mgen[:])
            mfirst_sb = misc.tile([128, 128], BF16, tag="mfirst")
            nc.sync.dma_start(out=mfirst_sb[:], in_=mfirst[:])

            for b in range(B):
                XT = big.tile([128, 8, SLAB], F32R, tag="xt")
                QT = big.tile([128, 8, SLICE], BF16, tag="qt")
                KT = big.tile([128, 8, SLAB], BF16, tag="kt")
                V1 = big.tile([128, NRT, NH, DH + 1], BF16, tag="v1")
                nc.vector.memset(V1[:, :, :, DH:DH + 1], 1.0)

                # --- Phase A: load + transpose X ---
                for rt in range(NRT):
                    xin = xin_pool.tile([128, HID], F32, tag="xin")
                    nc.sync.dma_start(out=xin[:],
                                      in_=x[b, 128 * rt:128 * rt + 128, :])
                    for ht in range(8):
                        tp = ps_small.tile([128, 128], F32, tag="small")
                        nc.tensor.transpose(
                            tp[:], xin[:, 128 * ht:128 * ht + 128], ident_sb[:])
                        nc.vector.tensor_copy(
                            XT[:, ht, 128 * rt:128 * rt + 128], tp[:])

                # --- Phase B: projections ---
                # QT: lhsT = wq tile [hid,outd], rhs = XT -> [outd, row] bf16
                for ot in range(8):
                    wts = []
                    for ht in range(8):
                        wt = wqk_pool.tile([128, 128], F32R, tag="wqk")
                        nc.sync.dma_start(
                            out=wt[:],
                            in_=wq[128 * ht:128 * ht + 128,
                                   128 * ot:128 * ot + 128])
                        wts.append(wt)
                    for half in range(2):
                        qp = ps_proj.tile([128, 512], F32, tag="proj")
                        for ht in range(8):
                            nc.tensor.matmul(
                                qp[:], wts[ht],
                                XT[:, ht, HALO + 512 * half:
                                   HALO + 512 * half + 512],
                                start=(ht == 0), stop=(ht == 7))
                        nc.vector.tensor_copy(
                            QT[:, ot, 512 * half:512 * half + 512], qp[:])

                # KT: same, over all SLAB cols, scaled by 1/sqrt(DH)
                for ot in range(8):
                    wts = []
                    for ht in range(8):
                        wt = wqk_pool.tile([128, 128], F32R, tag="wqk")
                        nc.sync.dma_start(
                            out=wt[:],
                            in_=wk[128 * ht:128 * ht + 128,
                                   128 * ot:128 * ot + 128])
                        wts.append(wt)
                    for ks in range(SLAB // KS):
                        kpf = ps_proj.tile([128, 512], F32, tag="proj",
                                           name="kpf")
                        kp = kpf[:, 0:KS]
                        for ht in range(8):
                            nc.tensor.matmul(
                                kp[:], wts[ht],
                                XT[:, ht, KS * ks:KS * ks + KS],
                                start=(ht == 0), stop=(ht == 7))
                        nc.vector.tensor_scalar_mul(
                            KT[:, ot, KS * ks:KS * ks + KS], kp[:],
                            1.0 / np.sqrt(DH))

                # V: lhsT = XT row tile, rhs = wv [hid, outd] -> [row, outd]
                for oh in range(2):
                    wvs = []
                    for ht in range(8):
                        wt = wv_pool.tile([128, 512], F32R, tag="wv")
                        nc.sync.dma_start(
                            out=wt[:],
                            in_=wv[128 * ht:128 * ht + 128,
                                   512 * oh:512 * oh + 512])
                        wvs.append(wt)
                    for rt in range(NRT):
                        vp = ps_proj.tile([128, 512], F32, tag="proj")
                        for ht in range(8):
                            nc.tensor.matmul(
                                vp[:], XT[:, ht, 128 * rt:128 * rt + 128],
                                wvs[ht], start=(ht == 0), stop=(ht == 7))
                        for hh in range(8):
                            nc.vector.tensor_copy(
                                V1[:, rt, 8 * oh + hh, 0:DH],
                                vp[:, 64 * hh:64 * hh + 64])

                # --- Phase C: attention ---
                for sp in range(NSP):
                    for t in range(NH // 2):
                        pts = {}
                        # band scores for both heads of the pair, per V tile
                        for l in range(5):
                            rt = 4 * sp + l
                            lo, hi = SPANS[l]
                            for e in range(2):
                                pp = ps_small.tile([128, 256], F32,
                                                   tag="small")
                                nc.tensor.matmul(
                                    pp[:, 0:hi - lo],
                                    KT[64 * e:64 * e + 64, t,
                                       128 * rt:128 * rt + 128],
                                    QT[64 * e:64 * e + 64, t,
                                       512 * sp + lo:512 * sp + hi],
                                    start=True, stop=True,
                                    tile_position=(64 * e, 0))
                                pt = pt_pool.tile([128, 256], BF16, tag="pt")
                                nc.scalar.activation(pt[:, 0:hi - lo],
                                                     pp[:, 0:hi - lo], EXP)
                                if l == 0 and sp == 0:
                                    msk = mfirst_sb[:]
                                else:
                                    ml, mh = MSLICE[l]
                                    msk = mgen_sb[:, ml:mh]
                                nc.vector.tensor_tensor(
                                    pt[:, 0:hi - lo], pt[:, 0:hi - lo], msk,
                                    mybir.AluOpType.mult)
                                pts[(e, l)] = pt
                        for e in range(2):
                            h = 2 * t + e
                            otA = ps_small.tile([DH + 1, 512], F32,
                                                tag="small", name="otA")
                            for l in (0, 2, 4):
                                rt = 4 * sp + l
                                lo, hi = SPANS[l]
                                nc.tensor.matmul(
                                    otA[:, lo:hi], V1[:, rt, h, :],
                                    pts[(e, l)][:, 0:hi - lo],
                                    start=(l == 0), stop=(l == 4))
                            otB = ps_small.tile([DH + 1, 512], F32,
                                                tag="small", name="otB")
                            for l in (1, 3):
                                rt = 4 * sp + l
                                lo, hi = SPANS[l]
                                nc.tensor.matmul(
                                    otB[:, lo:hi], V1[:, rt, h, :],
                                    pts[(e, l)][:, 0:hi - lo],
                                    start=(l == 1), stop=(l == 3))
                            otsb = ots_pool.tile([DH + 1, 512], F32, tag="osb")
                            nc.vector.tensor_copy(otsb[:], otA[:])
                            nc.vector.tensor_tensor(otsb[:], otsb[:], otB[:],
                                                    mybir.AluOpType.add)
                            for c4 in range(4):
                                trp = ps_small.tile([128, DH + 1], F32,
                                                    tag="small")
                                nc.tensor.transpose(
                                    trp[:], otsb[:, 128 * c4:128 * c4 + 128],
                                    ident_sb[0:DH + 1, 0:DH + 1])
                                rec = osb_pool.tile([128, 1], F32, tag="rec")
                                nc.vector.reciprocal(rec[:],
                                                     trp[:, DH:DH + 1])
                                osb = osb_pool.tile([128, DH], F32, tag="o")
                                nc.vector.tensor_scalar_mul(
                                    osb[:], trp[:, 0:DH], rec[:])
                                r0 = 512 * sp + 128 * c4
                                nc.sync.dma_start(
                                    out=out[b, r0:r0 + 128,
                                            DH * h:DH * h + DH],
                                    in_=osb[:])
    nc.compile()
    return nc


def _masks():
    """mbig [128, 320] = [D-1 | D0 | D1 | D2 | D3] where block Dd's two
    64-row halves are the masks for (qi_chunk - kv_chunk) = d and d-1:
    distance 0 -> causal (kv offset <= q offset), 1 -> all ones, else 0.
    Every per-tile mask the kernel needs is a contiguous slice of mbig."""
    causal = np.triu(np.ones((64, 64), dtype=np.float32))  # [kr, qr] kr<=qr
    ones = np.ones((64, 64), dtype=np.float32)
    zeros = np.zeros((64, 64), dtype=np.float32)

    def dblk(d):
        def m(dd):
            return causal if dd == 0 else (ones if dd == 1 else zeros)
        return np.concatenate([m(d), m(d - 1)], axis=0)  # [128, 64]

    gen = np.concatenate([dblk(d) for d in (-1, 0, 1, 2, 3)], axis=1)
    first = np.zeros((128, 128), dtype=np.float32)
    first[64:128, 0:64] = 1.0  # = mbig[:, 192:320]; all-zero on core 0
    return gen, first


def _inputs_for_core(i, hidden, wq, wk, wv):
    gen, first = _masks()
    if i == 0:
        first = np.zeros_like(first)
    idx = (np.arange(-HALO, SLICE) + SLICE * i) % S
    return {
        "x": np.ascontiguousarray(hidden[:, idx, :]),
        "wq": wq, "wk": wk, "wv": wv,
        "mgen": gen.astype(ml_dtypes.bfloat16),
        "mfirst": first.astype(ml_dtypes.bfloat16),
        "ident": np.eye(128, dtype=np.float32),
    }


def kernel(hidden_states, Wq, Wk, Wv, _trace=False):
    from concourse.bass_utils import run_bass_kernel_spmd

    hidden_states = np.asarray(hidden_states, dtype=np.float32)
    Wq = np.asarray(Wq, dtype=np.float32)
    Wk = np.asarray(Wk, dtype=np.float32)
    Wv = np.asarray(Wv, dtype=np.float32)

    if "nc" not in _CACHE:
        _CACHE["nc"] = _build()
    nc = _CACHE["nc"]

    in_maps = [_inputs_for_core(i, hidden_states, Wq, Wk, Wv)
               for i in range(CORES)]
    res = run_bass_kernel_spmd(nc, in_maps, list(range(CORES)), trace=_trace)
    _CACHE["last"] = res
    full = np.empty((B, S, HID), dtype=np.float32)
    for i in range(CORES):
        full[:, SLICE * i:SLICE * (i + 1), :] = res.results[i]["out"]
    return full
